# revision 1
# baseline (speedup 1.0000x reference)
"""Trainium2 Bass kernel for int4-grouped-quantized linear (GPTQ-style).

out[8192, 11008] = x[8192, 4096] @ dequant(qweight, qzeros, scales)

Sharding: column-parallel over out_features N across 8 NeuronCores.
Each core dequantizes its W shard [4096, 1376] on-chip (DVE), DMA-transposes
x chunks on the fly, and runs dense fp16 matmuls on the PE with fp32 PSUM
accumulation.
"""

import sys

sys.path.insert(0, "/opt/trn_rl_repo")

from contextlib import ExitStack

import numpy as np

import concourse.bass as bass
from concourse import bacc
import concourse.tile as tile
from concourse import mybir
from concourse.bass_utils import run_bass_kernel_spmd

AOT = mybir.AluOpType
F16, I32, F32 = mybir.dt.float16, mybir.dt.int32, mybir.dt.float32

T, K, N = 8192, 4096, 11008
NCORES = 8
NS = N // NCORES  # 1376 out cols per core
CS = NS // 8  # 172 packed int32 cols per core
G = 32  # quant groups (group size 128 == one k-block)
KB = K // 128  # 32 k-blocks
TCH = 512  # x rows per transpose chunk
NCHUNK = T // TCH  # 16
TBLK = TCH // 128  # 4 output row-blocks per chunk
SEGS = [(0, 512), (512, 512), (1024, 352)]  # N segments (PSUM bank sized)


def _body(ctx, tc, xd, qwd, qzd, scd, outd, zscr):
    nc = tc.nc
    cpool = ctx.enter_context(tc.tile_pool(name="const", bufs=1))
    qpool = ctx.enter_context(tc.tile_pool(name="qwp", bufs=4))
    stpool = ctx.enter_context(tc.tile_pool(name="stage", bufs=2))
    wpool = ctx.enter_context(tc.tile_pool(name="w", bufs=KB))
    bcpool = ctx.enter_context(tc.tile_pool(name="bc", bufs=3))
    xpool = ctx.enter_context(tc.tile_pool(name="x", bufs=3))
    xtpool = ctx.enter_context(tc.tile_pool(name="xt", bufs=KB + 8))
    tppool = ctx.enter_context(tc.tile_pool(name="tp", bufs=2, space="PSUM"))
    pspool = ctx.enter_context(tc.tile_pool(name="ps", bufs=2, space="PSUM"))
    opool = ctx.enter_context(tc.tile_pool(name="o", bufs=3))

    # ---- unpack zero-points: qz [G, CS] i32 -> z [G, NS] f16, park in DRAM ----
    qz_t = cpool.tile([G, CS], I32)
    nc.gpsimd.dma_start(qz_t[:], qzd)
    z_stage = cpool.tile([G, NS], I32)
    for j in range(8):
        nc.vector.tensor_scalar(
            z_stage[:, j::8], qz_t[:], 4 * j, 0xF,
            AOT.logical_shift_right, AOT.bitwise_and,
        )
    z_t = cpool.tile([G, NS], F16)
    nc.vector.tensor_copy(z_t[:], z_stage[:])
    nc.gpsimd.dma_start(zscr, z_t[:])

    # ---- dequantize W, one k-block (= one quant group) at a time ----
    w_tiles = []
    for b in range(KB):
        qw_t = qpool.tile([128, CS], I32)
        nc.gpsimd.dma_start(qw_t[:], qwd[b * 128 : (b + 1) * 128, :])
        w_stage = stpool.tile([128, NS], I32)
        for j in range(8):
            nc.vector.tensor_scalar(
                w_stage[:, j::8], qw_t[:], 4 * j, 0xF,
                AOT.logical_shift_right, AOT.bitwise_and,
            )
        w_t = wpool.tile([128, NS], F16)
        nc.scalar.copy(w_t[:], w_stage[:])
        # replicate this group's zero/scale row across 128 partitions via DMA
        z_bc = bcpool.tile([128, NS], F16, tag="zbc")
        nc.gpsimd.dma_start(z_bc[:], zscr[b : b + 1, :].partition_broadcast(128))
        s_bc = bcpool.tile([128, NS], F16, tag="sbc")
        nc.gpsimd.dma_start(s_bc[:], scd[b : b + 1, :].partition_broadcast(128))
        nc.vector.tensor_tensor(w_t[:], w_t[:], z_bc[:], AOT.subtract)
        nc.vector.tensor_tensor(w_t[:], w_t[:], s_bc[:], AOT.mult)
        w_tiles.append(w_t)

    # ---- identity for PE-mode transpose ----
    ident = cpool.tile([128, 128], F16)
    nc.gpsimd.memset(ident[:], 0.0)
    nc.gpsimd.affine_select(
        out=ident[:],
        in_=ident[:],
        compare_op=AOT.not_equal,
        fill=1.0,
        base=0,
        pattern=[[-1, 128]],
        channel_multiplier=1,
    )

    # ---- matmul: load x rows, PE-transpose 128x128 blocks, accumulate over K ----
    NT = T // 128  # 64 output row-blocks
    for t in range(NT):
        xsb = xpool.tile([128, K], F16, tag="xsb")
        nc.gpsimd.dma_start(xsb[:], xd[t * 128 : (t + 1) * 128, :])
        xts = []
        for b in range(KB):
            tp_ps = tppool.tile([128, 128], F16)
            nc.tensor.transpose(tp_ps[:], xsb[:, b * 128 : (b + 1) * 128], ident[:])
            xt = xtpool.tile([128, 128], F16, tag="xt")
            if b % 2 == 0:
                nc.vector.tensor_copy(xt[:], tp_ps[:])
            else:
                nc.scalar.copy(xt[:], tp_ps[:])
            xts.append(xt)
        ps = pspool.tile([128, NS], F32)
        for b in range(KB):
            for off, sz in SEGS:
                nc.tensor.matmul(
                    ps[:, off : off + sz],
                    xts[b][:],
                    w_tiles[b][:, off : off + sz],
                    start=(b == 0),
                    stop=(b == KB - 1),
                )
        ob = opool.tile([128, NS], F16)
        nc.any.tensor_copy(ob[:], ps[:])
        r0 = t * 128
        nc.gpsimd.dma_start(outd[r0 : r0 + 128, :], ob[:])


def build_kernel():
    nc = bacc.Bacc("TRN2", target_bir_lowering=False, debug=False)
    xd = nc.dram_tensor("x", [T, K], F16, kind="ExternalInput").ap()
    qwd = nc.dram_tensor("qw", [K, CS], I32, kind="ExternalInput").ap()
    qzd = nc.dram_tensor("qz", [G, CS], I32, kind="ExternalInput").ap()
    scd = nc.dram_tensor("sc", [G, NS], F16, kind="ExternalInput").ap()
    outd = nc.dram_tensor("out", [T, NS], F16, kind="ExternalOutput").ap()
    zscr = nc.dram_tensor("z_scratch", [G, NS], F16, kind="Internal").ap()
    with tile.TileContext(nc) as tc, ExitStack() as ctx:
        _body(ctx, tc, xd, qwd, qzd, scd, outd, zscr)
    nc.compile()
    return nc


_NC = None


def _get_nc():
    global _NC
    if _NC is None:
        _NC = build_kernel()
    return _NC


def make_in_maps(x, qweight, qzeros, scales):
    x = np.asarray(x, dtype=np.float16)
    qweight = np.asarray(qweight, dtype=np.int32)
    qzeros = np.asarray(qzeros, dtype=np.int32)
    scales = np.asarray(scales, dtype=np.float16)
    in_maps = []
    for c in range(NCORES):
        in_maps.append(
            {
                "x": x,
                "qw": np.ascontiguousarray(qweight[:, c * CS : (c + 1) * CS]),
                "qz": np.ascontiguousarray(qzeros[:, c * CS : (c + 1) * CS]),
                "sc": np.ascontiguousarray(scales[:, c * NS : (c + 1) * NS]),
            }
        )
    return in_maps


def run(in_maps, **kwargs):
    return run_bass_kernel_spmd(
        _get_nc(), in_maps, core_ids=list(range(NCORES)), **kwargs
    )


def kernel(x, qweight, qzeros, scales):
    res = run(make_in_maps(x, qweight, qzeros, scales))
    outs = [res.results[c]["out"] for c in range(NCORES)]
    return np.concatenate(outs, axis=1)

